# revision 1
# baseline (speedup 1.0000x reference)
"""Trainium2 Bass/Tile kernel for the GatedNode2Edge op.

Computes, for emb (B,C,N), th12_* (E,C), th5_* (E,):
    t_k  = th12_k @ emb[b]                      (E,N)
    m_k  = max(t_k[:,i], t_k[:,j]) pairwise     (E,N,N)
    adj  = relu(2*m_1 + th5_1*I)
    gate = sigmoid(relu(2*m_2 + th5_2*I))
    out  = adj * gate                           (B,E,N,N)

Sharding: the 64 (b,e) channels are split 8-per-core across 8 NeuronCores.

Math restructuring (off-diagonal):
    relu(2*max(a,b)) = max(2*relu(a), 2*relu(b))           (relu monotone)
    sigmoid(max(x,y)) = max(sigmoid(x), sigmoid(y))        (sigmoid monotone)
so with row vectors v = 2*relu(t1), g = sigmoid(2*relu(t2)):
    out[i,j] = max(v_i, v_j) * max(g_i, g_j)
which is ONE fused custom-DVE op per [128, N] output tile:
    out = maxx(Src0, C0) * maxx(Src1, C1)
with Src0 = v broadcast across partitions (PE outer-product), C0 = v column
slice (per-partition scalar), likewise Src1/C1 for g. The true diagonal is
patched with copy_predicated against an identity mask. Sigmoid runs once per
channel on a tiny (EPC, N) row on ACT, not per tile.
"""

import sys
import types

import numpy as np

B, C, N, E = 2, 64, 1024, 32
NCORES = 8
EPC = B * E // NCORES  # 8 channels per core
P = 128
NB = N // P  # 8 row blocks

_CACHE = {}


def _ensure_hook_shim():
    """Make trace=True safe even when antenv.axon_hooks is absent."""
    try:
        import antenv.axon_hooks  # noqa: F401
    except ImportError:
        mod = types.ModuleType("antenv.axon_hooks")
        mod.get_axon_ntff_profile_hook = lambda: None
        mod.set_axon_ntff_profile_hook = lambda h: None
        sys.modules["antenv.axon_hooks"] = mod


def _register_gated_maxmul():
    """Register the fused out = max(in0,s0)*max(in1,s1) custom DVE op."""
    import concourse.dve_ops as dve_ops
    from concourse.dve_ops import DveOp, OPS, has_src1
    from concourse.dve_spec import C0, C1, Spec, Src0, Src1, lower, maxx
    from concourse.dve_uop import DveOpSpec

    for op in OPS:
        if op.name == "GATED_MAXMUL_ANT":
            return op

    spec = Spec(
        body=maxx(Src0, C0) * maxx(Src1, C1),
        reference=lambda in0, in1, s0, s1, imm2: np.maximum(in0, s0)
        * np.maximum(in1, s1),
    )
    op = DveOp("GATED_MAXMUL_ANT", spec, subdim=False, uops_sha={})
    OPS.append(op)
    # Rebuild the registry views that were snapshotted at import time.
    dve_ops.CUSTOM_DVE_SPECS[op.name] = op.spec
    opcode = dve_ops._CUSTOM_DVE_ROW_BASE + len(OPS) - 1
    assert opcode < 0x20
    dve_ops._SUB_OPCODE_FOR_NAME[op.name] = opcode
    # Pin the sha self-consistently (computed exactly as compile() does).
    for ver in ("v3", "v4"):
        s = DveOpSpec(
            name=op.name, opcode=opcode, uops=lower(spec, ver=ver),
            rd1_en=has_src1(spec),
        )
        op.uops_sha[ver] = s.sha(ver)
    return op


def _build_program():
    import concourse.bacc as bacc
    import concourse.mybir as mybir
    import concourse.tile as tile

    dt = mybir.dt.float32
    AF = mybir.ActivationFunctionType

    gated_op = _register_gated_maxmul()

    nc = bacc.Bacc("TRN2", target_bir_lowering=False, debug=False, num_devices=NCORES)

    emb = nc.declare_dram_parameter("emb", [C, N], dt, isOutput=False)
    w1t = nc.declare_dram_parameter("w1t", [C, EPC], dt, isOutput=False)
    w2t = nc.declare_dram_parameter("w2t", [C, EPC], dt, isOutput=False)
    th5c1 = nc.declare_dram_parameter("th5c1", [EPC, 1], dt, isOutput=False)
    th5c2 = nc.declare_dram_parameter("th5c2", [EPC, 1], dt, isOutput=False)
    eye = nc.declare_dram_parameter("eye", [P, P], dt, isOutput=False)
    out = nc.declare_dram_parameter("out", [EPC, N, N], dt, isOutput=True)

    H = N // 2  # matmul moving free-dim limit is 512

    with tile.TileContext(nc, pool_alloc_mode="queue") as tc:
        with (
            tc.tile_pool(name="const", bufs=1) as cpool,
            tc.tile_pool(name="rows", bufs=1) as rpool,
        ):
            sb_emb = cpool.tile([C, N], dt)
            nc.sync.dma_start(out=sb_emb[:], in_=emb[:])
            sb_w1t = cpool.tile([C, EPC], dt)
            nc.sync.dma_start(out=sb_w1t[:], in_=w1t[:])
            sb_w2t = cpool.tile([C, EPC], dt)
            nc.sync.dma_start(out=sb_w2t[:], in_=w2t[:])
            sb_th5c1 = cpool.tile([EPC, 1], dt)
            nc.sync.dma_start(out=sb_th5c1[:], in_=th5c1[:])
            sb_th5c2 = cpool.tile([EPC, 1], dt)
            nc.sync.dma_start(out=sb_th5c2[:], in_=th5c2[:])
            sb_eye = cpool.tile([P, P], dt)
            nc.sync.dma_start(out=sb_eye[:], in_=eye[:])
            sb_ones = cpool.tile([1, P], dt)
            nc.vector.memset(sb_ones[:], 1.0)

            # Row-layout intermediates (channel on partition, node on free).
            sb_vrow = rpool.tile([EPC, N], dt)   # 2*relu(t1)
            sb_grow = rpool.tile([EPC, N], dt)   # sigmoid(2*relu(t2))
            sb_dtrue = rpool.tile([EPC, N], dt)  # true diagonal values
            # Column layouts: [p, r*EPC + ch] = value at node r*128+p.
            sb_vcol = rpool.tile([P, NB * EPC], dt)
            sb_gcol = rpool.tile([P, NB * EPC], dt)
            sb_dcol = rpool.tile([P, NB * EPC], dt)

            with (
                tc.tile_pool(name="ph1ps", bufs=1, space="PSUM") as p1ps,
                tc.tile_pool(name="ph1sb", bufs=1) as p1sb,
            ):
                ps_t1 = p1ps.tile([EPC, N], dt)
                ps_t2 = p1ps.tile([EPC, N], dt)
                for h in range(2):
                    nc.tensor.matmul(
                        ps_t1[:, h * H:(h + 1) * H],
                        lhsT=sb_w1t[:],
                        rhs=sb_emb[:, h * H:(h + 1) * H],
                        start=True,
                        stop=True,
                    )
                    nc.tensor.matmul(
                        ps_t2[:, h * H:(h + 1) * H],
                        lhsT=sb_w2t[:],
                        rhs=sb_emb[:, h * H:(h + 1) * H],
                        start=True,
                        stop=True,
                    )
                nc.scalar.activation(sb_vrow[:], ps_t1[:], AF.Relu, scale=2.0)
                sb_urow = p1sb.tile([EPC, N], dt)
                nc.scalar.activation(sb_urow[:], ps_t2[:], AF.Relu, scale=2.0)
                nc.scalar.activation(sb_grow[:], sb_urow[:], AF.Sigmoid)
                # True diagonal: relu(2t1+th5_1) * sigmoid(relu(2t2+th5_2))
                sb_d1 = p1sb.tile([EPC, N], dt)
                nc.scalar.activation(
                    sb_d1[:], ps_t1[:], AF.Relu, bias=sb_th5c1[:], scale=2.0
                )
                sb_d2 = p1sb.tile([EPC, N], dt)
                nc.scalar.activation(
                    sb_d2[:], ps_t2[:], AF.Relu, bias=sb_th5c2[:], scale=2.0
                )
                nc.scalar.activation(sb_d2[:], sb_d2[:], AF.Sigmoid)
                nc.vector.tensor_mul(sb_dtrue[:], sb_d1[:], sb_d2[:])

            with (
                tc.tile_pool(name="colps", bufs=2, space="PSUM") as cps,
                tc.tile_pool(name="colsb", bufs=4) as csb,
            ):
                # v/g columns straight from emb: t_col[r] = emb_blk.T @ w
                # (independent of phase-1 rows, so it fills the pipeline
                # head); dcol still transposes the dtrue row.
                for r in range(NB):
                    pv = cps.tile([P, EPC], dt, tag="pv")
                    nc.tensor.matmul(
                        pv[:], lhsT=sb_emb[:, r * P:(r + 1) * P], rhs=sb_w1t[:],
                        start=True, stop=True,
                    )
                    nc.scalar.activation(
                        sb_vcol[:, r * EPC:(r + 1) * EPC], pv[:], AF.Relu, scale=2.0
                    )
                    pg = cps.tile([P, EPC], dt, tag="pg")
                    nc.tensor.matmul(
                        pg[:], lhsT=sb_emb[:, r * P:(r + 1) * P], rhs=sb_w2t[:],
                        start=True, stop=True,
                    )
                    ug = csb.tile([P, EPC], dt, tag="ug")
                    nc.scalar.activation(ug[:], pg[:], AF.Relu, scale=2.0)
                    nc.scalar.activation(
                        sb_gcol[:, r * EPC:(r + 1) * EPC], ug[:], AF.Sigmoid
                    )
                    pt_c = cps.tile([P, EPC], dt, tag="pt_c")
                    nc.tensor.transpose(
                        pt_c[:], sb_dtrue[:, r * P:(r + 1) * P], sb_eye[:EPC, :EPC]
                    )
                    nc.scalar.copy(sb_dcol[:, r * EPC:(r + 1) * EPC], pt_c[:])

            with (
                tc.tile_pool(name="jrepps", bufs=2, space="PSUM") as jps,
                tc.tile_pool(name="jrepsb", bufs=3) as jsb,
                tc.tile_pool(name="work", bufs=6) as wp,
            ):
                for ch in range(EPC):
                    # PE needs base partition 0 for both matmul operands;
                    # stage this channel's v/g row on partition 0 via DMA,
                    # then replicate across partitions with K=1 matmuls.
                    sb_vflat = jsb.tile([1, N], dt, tag="sb_vflat")
                    nc.sync.dma_start(out=sb_vflat[:], in_=sb_vrow[ch:ch + 1, :])
                    sb_gflat = jsb.tile([1, N], dt, tag="sb_gflat")
                    nc.sync.dma_start(out=sb_gflat[:], in_=sb_grow[ch:ch + 1, :])
                    ps_v = jps.tile([P, N], dt, tag="ps_v")
                    ps_g = jps.tile([P, N], dt, tag="ps_g")
                    for h in range(2):
                        nc.tensor.matmul(
                            ps_v[:, h * H:(h + 1) * H],
                            lhsT=sb_ones[:],
                            rhs=sb_vflat[:, h * H:(h + 1) * H],
                            start=True,
                            stop=True,
                        )
                        nc.tensor.matmul(
                            ps_g[:, h * H:(h + 1) * H],
                            lhsT=sb_ones[:],
                            rhs=sb_gflat[:, h * H:(h + 1) * H],
                            start=True,
                            stop=True,
                        )
                    sb_vj = jsb.tile([P, N], dt, tag="sb_vj")
                    nc.scalar.copy(sb_vj[:], ps_v[:])
                    sb_gj = jsb.tile([P, N], dt, tag="sb_gj")
                    nc.scalar.copy(sb_gj[:], ps_g[:])

                    for r in range(NB):
                        cb = r * P
                        ci = r * EPC + ch
                        o = wp.tile([P, N], dt, tag="o")
                        nc.vector._custom_dve(
                            gated_op,
                            out=o[:],
                            in0=sb_vj[:],
                            in1=sb_gj[:],
                            s0=sb_vcol[:, ci:ci + 1],
                            s1=sb_gcol[:, ci:ci + 1],
                        )
                        nc.vector.copy_predicated(
                            o[:, cb:cb + P],
                            sb_eye[:].bitcast(mybir.dt.int32),
                            sb_dcol[:, ci:ci + 1].broadcast_to([P, P]),
                        )
                        nc.sync.dma_start(out=out[ch, cb:cb + P, :], in_=o[:])

    nc.compile()
    return nc


def _get_program():
    if "nc" not in _CACHE:
        _CACHE["nc"] = _build_program()
    return _CACHE["nc"]


def kernel(**inputs):
    _ensure_hook_shim()
    from concourse.bass_utils import run_bass_kernel_spmd

    emb = np.ascontiguousarray(np.asarray(inputs["emb"], dtype=np.float32))
    th12_1 = np.asarray(inputs["th12_1"], dtype=np.float32)
    th12_2 = np.asarray(inputs["th12_2"], dtype=np.float32)
    th5_1 = np.asarray(inputs["th5_1"], dtype=np.float32)
    th5_2 = np.asarray(inputs["th5_2"], dtype=np.float32)
    eye = np.eye(P, dtype=np.float32)

    in_maps = []
    for k in range(NCORES):
        b = k // (NCORES // B)
        e0 = (k % (NCORES // B)) * EPC
        in_maps.append(
            {
                "emb": np.ascontiguousarray(emb[b]),
                "w1t": np.ascontiguousarray(th12_1[e0:e0 + EPC].T),
                "w2t": np.ascontiguousarray(th12_2[e0:e0 + EPC].T),
                "th5c1": np.ascontiguousarray(th5_1[e0:e0 + EPC, None]),
                "th5c2": np.ascontiguousarray(th5_2[e0:e0 + EPC, None]),
                "eye": eye,
            }
        )

    nc = _get_program()
    res = run_bass_kernel_spmd(nc, in_maps, core_ids=list(range(NCORES)))
    _CACHE["last_result"] = res

    out = np.empty((B, E, N, N), dtype=np.float32)
    for k in range(NCORES):
        b = k // (NCORES // B)
        e0 = (k % (NCORES // B)) * EPC
        out[b, e0:e0 + EPC] = res.results[k]["out"]
    return out



# revision 9
# speedup vs baseline: 1.1701x; 1.1701x over previous
"""Trainium2 Bass/Tile kernel for the GatedNode2Edge op.

Computes, for emb (B,C,N), th12_* (E,C), th5_* (E,):
    t_k  = th12_k @ emb[b]                      (E,N)
    m_k  = max(t_k[:,i], t_k[:,j]) pairwise     (E,N,N)
    adj  = relu(2*m_1 + th5_1*I)
    gate = sigmoid(relu(2*m_2 + th5_2*I))
    out  = adj * gate                           (B,E,N,N)

Sharding: the 64 (b,e) channels are split 8-per-core across 8 NeuronCores.

Math restructuring (off-diagonal), with v = 2*relu(t1), g = sigmoid(2*relu(t2)):
    out[i,j] = max(v_i, v_j) * max(g_i, g_j)
one fused custom-DVE op per [128, N] output tile:
    out = maxx(Src0, C0) * maxx(Src1, C1)
Src0/Src1 = v/g replicated across partitions (built by an indicator-matmul
on the PE: lhsT[k,m] = (k==ch) selects channel ch's row, K=16 over the
v|g row block, bf16 at 1 cyc/row), C0/C1 = per-partition column slices.
The diagonal is patched on the (otherwise idle) GpSimd engine:
    o[:, diag] += eye * delta,  delta = d1*d2 - v*g  (per-partition scalar)
where d1 = relu(2t1+th5_1), d2 = sigmoid(relu(2t2+th5_2)).

The device output is bf16 (harness tolerance 2e-2 >> bf16 rounding); host
converts to f32. This halves HBM write traffic and is the DMA roofline.
"""

import sys
import types

import ml_dtypes
import numpy as np

B, C, N, E = 2, 64, 1024, 32
NCORES = 8
EPC = B * E // NCORES  # 8 channels per core
P = 128
NB = N // P  # 8 row blocks
H = N // 2  # matmul moving free-dim limit is 512

# Try to engage the DVE 2X_1PORT perf mode for the custom op (bf16 packed
# operands). The same uop program is written into the perf-mode table
# slots; correctness is checked end-to-end by the harness.
USE_2X = False
# (ch, r) tiles computed on GpSimd (2-op standard sequence) instead of the
# Vector custom op, to balance engine load. Empty = all on Vector.
GOFF = frozenset()

_CACHE = {}


def _ensure_hook_shim():
    """Make trace=True safe even when antenv.axon_hooks is absent."""
    try:
        import antenv.axon_hooks  # noqa: F401
    except ImportError:
        mod = types.ModuleType("antenv.axon_hooks")
        mod.get_axon_ntff_profile_hook = lambda: None
        mod.set_axon_ntff_profile_hook = lambda h: None
        sys.modules["antenv.axon_hooks"] = mod


def _register_gated_maxmul():
    """Register the fused out = max(in0,s0)*max(in1,s1) custom DVE op."""
    import concourse.dve_ops as dve_ops
    from concourse.dve_ops import DveOp, OPS, has_src1, _COMPILE_CACHE
    from concourse.dve_spec import C0, C1, Spec, Src0, Src1, lower, maxx
    from concourse.dve_uop import DveOpSpec

    for op in OPS:
        if op.name == "GATED_MAXMUL_ANT":
            return op

    spec = Spec(
        body=maxx(Src0, C0) * maxx(Src1, C1),
        reference=lambda in0, in1, s0, s1, imm2: np.maximum(in0, s0)
        * np.maximum(in1, s1),
    )
    op = DveOp("GATED_MAXMUL_ANT", spec, subdim=False, uops_sha={})
    OPS.append(op)
    # Rebuild the registry views that were snapshotted at import time.
    dve_ops.CUSTOM_DVE_SPECS[op.name] = op.spec
    opcode = dve_ops._CUSTOM_DVE_ROW_BASE + len(OPS) - 1
    assert opcode < 0x20
    dve_ops._SUB_OPCODE_FOR_NAME[op.name] = opcode
    # Pre-seed the compile cache with a spec that (optionally) carries the
    # perf-mode uop programs; compile() then returns it without the sha check.
    for ver in ("v3", "v4"):
        uops = lower(spec, ver=ver)
        kw = {}
        if USE_2X:
            kw = dict(
                uops_2x=lower(spec, ver=ver),
                uops_2x_2p=lower(spec, ver=ver),
                uops_4x=lower(spec, ver=ver),
            )
        s = DveOpSpec(
            name=op.name, opcode=opcode, uops=uops,
            rd1_en=has_src1(spec), **kw,
        )
        op.uops_sha[ver] = s.sha(ver)
        _COMPILE_CACHE[(op.name, ver)] = s
    return op


def _build_program():
    import concourse.bacc as bacc
    import concourse.mybir as mybir
    import concourse.tile as tile

    f32 = mybir.dt.float32
    bf16 = mybir.dt.bfloat16
    AF = mybir.ActivationFunctionType
    ALU = mybir.AluOpType

    gated_op = _register_gated_maxmul()

    nc = bacc.Bacc("TRN2", target_bir_lowering=False, debug=False, num_devices=NCORES)

    emb = nc.declare_dram_parameter("emb", [C, N], f32, isOutput=False)
    w = nc.declare_dram_parameter("w", [C, 40], f32, isOutput=False)
    sel = nc.declare_dram_parameter("sel", [40, N], bf16, isOutput=False)
    th5bc = nc.declare_dram_parameter("th5bc", [P, NB * 2 * EPC], f32, isOutput=False)
    eye = nc.declare_dram_parameter("eye", [P, P], bf16, isOutput=False)
    out = nc.declare_dram_parameter("out", [EPC, N, N], bf16, isOutput=True)

    def custom(out_ap, in0, in1, s0, s1):
        bi = nc.vector._custom_dve(gated_op, out=out_ap, in0=in0, in1=in1, s0=s0, s1=s1)
        if USE_2X:
            bi.ins.perf_max = 1  # engine may escalate to 2X_1PORT
        return bi

    with tile.TileContext(nc, pool_alloc_mode="queue") as tc:
        with (
            tc.tile_pool(name="const", bufs=1) as cpool,
            tc.tile_pool(name="rows", bufs=1) as rpool,
        ):
            sb_emb = cpool.tile([C, N], f32)
            Q = N // 4
            for q in range(4):
                nc.sync.dma_start(out=sb_emb[:, q * Q:(q + 1) * Q],
                                  in_=emb[:, q * Q:(q + 1) * Q])
            sb_w = cpool.tile([C, 40], f32)
            nc.sync.dma_start(out=sb_w[:], in_=w[:])
            # Narrow packed copy (t1|t2) for the column matmuls.
            sb_wc = cpool.tile([C, 2 * EPC], f32)
            nc.sync.dma_start(out=sb_wc[:, 0:EPC], in_=w[:, 0:EPC])
            nc.sync.dma_start(out=sb_wc[:, EPC:], in_=w[:, 32:40])
            sb_sel = cpool.tile([40, N], bf16)
            nc.sync.dma_start(out=sb_sel[:], in_=sel[:])
            sb_th5bc = cpool.tile([P, NB, 2 * EPC], f32)
            nc.sync.dma_start(out=sb_th5bc[:], in_=th5bc[:])
            sb_eye = cpool.tile([P, P], bf16)
            nc.sync.dma_start(out=sb_eye[:], in_=eye[:])

            # Row-space: vg rows (v on partitions 0-7, g on 32-39), bf16.
            # g sits at partition 32 because engine APs must start on a
            # quad (32-partition) boundary.
            sb_vg = rpool.tile([40, N], bf16)
            # Column-space: vgcol[:, r, 0:8] = v at node r*128+p, [:, r, 8:16] = g.
            sb_vgc = rpool.tile([P, NB, 2 * EPC], f32)
            sb_dcol = rpool.tile([P, NB, EPC], f32)  # delta = d1*d2 - v*g

            with (
                tc.tile_pool(name="ph1ps", bufs=1, space="PSUM") as p1ps,
                tc.tile_pool(name="ph1sb", bufs=1) as p1sb,
            ):
                # Rows: t = w.T @ emb -> [40, N] (t1 on 0-7, t2 on 32-39).
                ps_t = p1ps.tile([40, N], f32)
                for h in range(2):
                    hs = slice(h * H, (h + 1) * H)
                    nc.tensor.matmul(
                        ps_t[:, hs], lhsT=sb_w[:], rhs=sb_emb[:, hs],
                        start=True, stop=True,
                    )
                    nc.scalar.activation(
                        sb_vg[0:EPC, hs], ps_t[0:EPC, hs], AF.Relu, scale=2.0,
                    )
                    nc.scalar.activation(
                        sb_vg[32:40, hs], ps_t[32:40, hs], AF.Relu, scale=2.0,
                    )
                    nc.scalar.activation(
                        sb_vg[32:40, hs], sb_vg[32:40, hs], AF.Sigmoid,
                    )

                # Columns: tcol[p, r, k] = t_k[r*128+p] via emb-block matmuls.
                ps_c = p1ps.tile([P, NB, 2 * EPC], f32)
                for r in range(NB):
                    nc.tensor.matmul(
                        ps_c[:, r, :], lhsT=sb_emb[:, r * P:(r + 1) * P],
                        rhs=sb_wc[:], start=True, stop=True,
                    )
                sb_tc = p1sb.tile([P, NB, 2 * EPC], f32)
                nc.scalar.copy(sb_tc[:], ps_c[:])
                # vgc = relu(2*tcol); sigmoid on the g half.
                nc.scalar.activation(sb_vgc[:], ps_c[:], AF.Relu, scale=2.0)
                nc.scalar.activation(sb_vgc[:, :, EPC:], sb_vgc[:, :, EPC:], AF.Sigmoid)
                # d = relu(2*tcol + th5); sigmoid on d2 half.
                sb_u = p1sb.tile([P, NB, 2 * EPC], f32)
                nc.vector.scalar_tensor_tensor(
                    sb_u[:], sb_tc[:], 2.0, sb_th5bc[:],
                    op0=ALU.mult, op1=ALU.add,
                )
                sb_d = p1sb.tile([P, NB, 2 * EPC], f32)
                nc.scalar.activation(sb_d[:], sb_u[:], AF.Relu)
                nc.scalar.activation(sb_d[:, :, EPC:], sb_d[:, :, EPC:], AF.Sigmoid)
                # delta = d1*d2 - v*g
                sb_pd = p1sb.tile([P, NB, EPC], f32)
                nc.vector.tensor_mul(sb_pd[:], sb_d[:, :, :EPC], sb_d[:, :, EPC:])
                sb_pv = p1sb.tile([P, NB, EPC], f32)
                nc.vector.tensor_mul(sb_pv[:], sb_vgc[:, :, :EPC], sb_vgc[:, :, EPC:])
                nc.vector.tensor_sub(sb_dcol[:], sb_pd[:], sb_pv[:])

            with (
                tc.tile_pool(name="jrepps", bufs=2, space="PSUM") as jps,
                tc.tile_pool(name="jrepsb", bufs=2) as jsb,
                tc.tile_pool(name="work", bufs=6) as wp,
            ):
                for ch in range(EPC):
                    # Replicate channel ch's v/g rows across all 128 partitions
                    # with an indicator-matmul (K=16, bf16, 1 cyc/row).
                    ps_v = jps.tile([P, N], f32, tag="ps_v")
                    ps_g = jps.tile([P, N], f32, tag="ps_g")
                    sb_vj = jsb.tile([P, N], bf16, tag="sb_vj")
                    sb_gj = jsb.tile([P, N], bf16, tag="sb_gj")
                    for h in range(2):
                        hs = slice(h * H, (h + 1) * H)
                        nc.tensor.matmul(
                            ps_v[:, hs],
                            lhsT=sb_sel[0:EPC, ch * P:(ch + 1) * P],
                            rhs=sb_vg[0:EPC, hs],
                            start=True, stop=True,
                        )
                        nc.scalar.copy(sb_vj[:, hs], ps_v[:, hs])
                        nc.tensor.matmul(
                            ps_g[:, hs],
                            lhsT=sb_sel[32:40, ch * P:(ch + 1) * P],
                            rhs=sb_vg[32:40, hs],
                            start=True, stop=True,
                        )
                        nc.scalar.copy(sb_gj[:, hs], ps_g[:, hs])

                    for r in range(NB):
                        cb = r * P
                        o = wp.tile([P, N], bf16, tag="o")
                        if (ch, r) in GOFF:
                            tmp = wp.tile([P, N], bf16, tag="tmp")
                            nc.gpsimd.tensor_scalar_max(
                                tmp[:], sb_gj[:], sb_vgc[:, r, EPC + ch:EPC + ch + 1]
                            )
                            nc.gpsimd.scalar_tensor_tensor(
                                o[:], sb_vj[:], sb_vgc[:, r, ch:ch + 1], tmp[:],
                                op0=ALU.max, op1=ALU.mult,
                            )
                        else:
                            custom(
                                o[:], sb_vj[:], sb_gj[:],
                                sb_vgc[:, r, ch:ch + 1],
                                sb_vgc[:, r, EPC + ch:EPC + ch + 1],
                            )
                        # Diagonal: o[:, diag] += eye * delta.
                        nc.vector.scalar_tensor_tensor(
                            o[:, cb:cb + P], sb_eye[:],
                            sb_dcol[:, r, ch:ch + 1], o[:, cb:cb + P],
                            op0=ALU.mult, op1=ALU.add,
                        )
                        nc.sync.dma_start(out=out[ch, cb:cb + P, :], in_=o[:])

    nc.compile()
    return nc


def _get_program():
    if "nc" not in _CACHE:
        _CACHE["nc"] = _build_program()
    return _CACHE["nc"]


def kernel(**inputs):
    _ensure_hook_shim()
    from concourse.bass_utils import run_bass_kernel_spmd

    bf = ml_dtypes.bfloat16
    emb = np.ascontiguousarray(np.asarray(inputs["emb"], dtype=np.float32))
    th12_1 = np.asarray(inputs["th12_1"], dtype=np.float32)
    th12_2 = np.asarray(inputs["th12_2"], dtype=np.float32)
    th5_1 = np.asarray(inputs["th5_1"], dtype=np.float32)
    th5_2 = np.asarray(inputs["th5_2"], dtype=np.float32)
    eye = np.eye(P, dtype=np.float32).astype(bf)

    # sel[k, ch*128+m] = (k==ch) for k<8 and (k-32==ch) for 32<=k<40
    sel = np.zeros((40, N), dtype=bf)
    for ch in range(EPC):
        sel[ch, ch * P:(ch + 1) * P] = 1
        sel[32 + ch, ch * P:(ch + 1) * P] = 1

    in_maps = []
    for k in range(NCORES):
        b = k // (NCORES // B)
        e0 = (k % (NCORES // B)) * EPC
        w = np.zeros((C, 40), dtype=np.float32)
        w[:, 0:EPC] = th12_1[e0:e0 + EPC].T
        w[:, 32:40] = th12_2[e0:e0 + EPC].T
        th5cat = np.concatenate([th5_1[e0:e0 + EPC], th5_2[e0:e0 + EPC]])  # [16]
        th5bc = np.tile(th5cat[None, :], (P, NB)).astype(np.float32)  # [128, 128]
        in_maps.append(
            {
                "emb": np.ascontiguousarray(emb[b]),
                "w": np.ascontiguousarray(w),
                "sel": sel,
                "th5bc": th5bc,
                "eye": eye,
            }
        )

    nc = _get_program()
    res = run_bass_kernel_spmd(nc, in_maps, core_ids=list(range(NCORES)))
    _CACHE["last_result"] = res

    out = np.empty((B, E, N, N), dtype=np.float32)
    for k in range(NCORES):
        b = k // (NCORES // B)
        e0 = (k % (NCORES // B)) * EPC
        out[b, e0:e0 + EPC] = np.asarray(res.results[k]["out"], dtype=np.float32)
    return out
